# revision 23
# baseline (speedup 1.0000x reference)
"""Trainium2 Bass kernel for KeystrokeAttention.

Math: context[b] = softmax_s(hidden[b].Wh + enc[b,s].We + bias) @ enc[b]
Softmax is shift-invariant, and (hidden[b].Wh + bias) is constant over the
softmax axis s, so it cancels exactly: context[b] = softmax_s(enc[b,s].We) @ enc[b].
Only encoder_outputs (256 MiB) and W_e (4 KB) are needed on device.

Sharding: data-parallel over batch. B=32 across 8 cores -> 4 batches/core.
Per core: read 32 MiB of enc once (memory roofline ~93 us @ 358 GB/s).

Per batch b (S=2048 split into 16 s-tiles of 128 partitions x 1024):
  1. DMA s-tile t -> SBUF [128, 1024] (natural layout, s on partitions)
  2. DVE tensor_tensor_reduce: E[:, t] = sum_h enc_tile * We_bcast (fused mul+reduce)
  3. ACT exp: P = exp(E) [128, 16], accum_out srow = per-partition sums
     (no max subtraction: energies are O(1) for randn inputs; softmax is
     mathematically identical with any shift)
  4. PE: Z = srow^T @ ones  (cross-partition sum, [1,1] PSUM)
  5. PE context matmuls, variant-dependent (see below)
  6. ACT: scale by 1/Z, DMA out
"""

import os
import sys

for _p in ("/opt/trn_rl_repo", "/root/.axon_site/_ro/trn_rl_repo"):
    if os.path.isdir(_p) and _p not in sys.path:
        sys.path.insert(0, _p)

import numpy as np

B = 32
S = 2048
H = 1024
NCORES = 8
BLOC = B // NCORES  # 4 batches per core
P = 128
NT = S // P  # 16 s-tiles per batch
NCH = H // P  # 8 h-chunks of 128

# "f32_mov": attn-stationary fp32 matmuls, N=512 moving operand (exact fp32
#            numerics; PE hides fully under the DMA roofline) -- default
# "f32_stat": enc-stationary fp32 matmuls (exact but slow: N=1 partial-line
#             PSUM writes + per-matmul LDWEIGHTS churn)
# "f32r": attn-stationary float32r (rejected: walrus compile failure here)
VARIANT = os.environ.get("KA_VARIANT", "f32_mov")
# debug controls: restrict pipeline stages / batch count for HW bisection
STAGE = os.environ.get("KA_STAGE", "full")  # dma | ttr | exp | ctx | full
DBG_BLOC = int(os.environ.get("KA_BLOC", str(BLOC)))
# ttr/stt (fused DVE accum_out) crash HW on this platform; tt_act works.
ENERGY = os.environ.get("KA_ENERGY", "tt_act")  # ttr | stt | tt_act
# timing-only: repeat the whole batch pipeline R times inside one NEFF via a
# Tile For_i loop, so per-iteration HW time can be extracted by differencing
# two R values (the axon dispatch floor is ~77 ms and cancels out).
REPEAT = int(os.environ.get("KA_REPEAT", "0"))
# DMA issue strategy for the big enc loads: which engines post the dma_starts
DMA_MODE = os.environ.get("KA_DMA", "sync")  # sync | dual | gpsimd | mix
ENC_BUFS = int(os.environ.get("KA_ENC_BUFS", "40"))

_CACHE = {}


def _build(variant):
    import concourse.bacc as bacc
    import concourse.tile as tile
    from concourse import mybir

    f32 = mybir.dt.float32
    f32r = mybir.dt.float32r
    Alu = mybir.AluOpType
    Act = mybir.ActivationFunctionType

    nc = bacc.Bacc(
        "TRN2",
        target_bir_lowering=False,
        debug=False,
        num_devices=NCORES,
    )

    nbat = DBG_BLOC
    enc_t = nc.dram_tensor("enc", [BLOC, S, H], f32, kind="ExternalInput")
    we_t = nc.dram_tensor("we", [1, H], f32, kind="ExternalInput")
    if variant == "f32_stat":
        # out[b, c, p, 0] = context[b, c*128 + p]
        out_t = nc.dram_tensor("out", [BLOC, NCH, P, 1], f32, kind="ExternalOutput")
    else:
        out_t = nc.dram_tensor("out", [BLOC, H], f32, kind="ExternalOutput")

    enc = enc_t.ap()
    we = we_t.ap()
    out = out_t.ap()

    with tile.TileContext(nc) as tc:
        with (
            tc.tile_pool(name="consts", bufs=1) as consts,
            tc.tile_pool(name="encp", bufs=ENC_BUFS) as encp,
            tc.tile_pool(name="work", bufs=2) as work,
            tc.tile_pool(name="small", bufs=3) as small,
            tc.tile_pool(name="psc", bufs=2, space="PSUM") as psum_ctx,
            tc.tile_pool(name="psm", bufs=2, space="PSUM") as psum_misc,
        ):
            we_b = consts.tile([P, H], f32)
            nc.gpsimd.dma_start(out=we_b, in_=we.to_broadcast([P, H]))
            ones_col = consts.tile([P, 1], f32)
            nc.vector.memset(ones_col, 1.0)
            ones_row = consts.tile([1, P], f32)
            nc.vector.memset(ones_row, 1.0)

            _rep = None
            if REPEAT > 0:
                _rep = tc.For_i(0, REPEAT, 1, name="rep")
                _rep.__enter__()
            if DMA_MODE == "sync":
                dma_engs = [nc.sync]
            elif DMA_MODE == "dual":
                dma_engs = [nc.sync, nc.scalar]
            elif DMA_MODE == "gpsimd":
                dma_engs = [nc.gpsimd]
            else:  # mix
                dma_engs = [nc.sync, nc.scalar, nc.gpsimd]

            for b in range(nbat):
                ets = []
                for t in range(NT):
                    et = encp.tile([P, H], f32, tag="enc")
                    eng = dma_engs[t % len(dma_engs)]
                    eng.dma_start(out=et, in_=enc[b, t * P : (t + 1) * P, :])
                    ets.append(et)

                if STAGE == "dma":
                    ctx_sb = small.tile([P, NCH], f32, tag="ctx_sb")
                    nc.scalar.copy(ctx_sb, ets[0][:, :NCH])
                    for c in range(NCH):
                        nc.sync.dma_start(out=out[b, c], in_=ctx_sb[:, c : c + 1])
                    continue

                E = small.tile([P, NT], f32, tag="E")
                for t in range(NT):
                    prod = work.tile([P, H], f32, tag="prod")
                    if ENERGY == "ttr":
                        nc.vector.tensor_tensor_reduce(
                            out=prod,
                            in0=ets[t],
                            in1=we_b,
                            scale=1.0,
                            scalar=0.0,
                            op0=Alu.mult,
                            op1=Alu.add,
                            accum_out=E[:, t : t + 1],
                        )
                    elif ENERGY == "stt":
                        # out = (in0 bypass scalar) * in1 ; accum_out = row sums
                        nc.vector.scalar_tensor_tensor(
                            out=prod,
                            in0=ets[t],
                            scalar=1.0,
                            in1=we_b,
                            op0=Alu.bypass,
                            op1=Alu.mult,
                            accum_out=E[:, t : t + 1],
                        )
                    else:  # "tt_act": DVE multiply, ACT accumulate-reduce
                        nc.vector.tensor_tensor(
                            out=prod, in0=ets[t], in1=we_b, op=Alu.mult
                        )
                        psink = work.tile([P, H], f32, tag="psink")
                        nc.scalar.activation(
                            out=psink,
                            in_=prod,
                            func=Act.Copy,
                            accum_out=E[:, t : t + 1],
                        )

                if STAGE == "ttr":
                    ctx_sb = small.tile([P, NCH], f32, tag="ctx_sb")
                    nc.scalar.copy(ctx_sb, E[:, :NCH])
                    for c in range(NCH):
                        nc.sync.dma_start(out=out[b, c], in_=ctx_sb[:, c : c + 1])
                    continue

                if STAGE == "ctx":
                    # context matmuls fed by raw E (skips exp/Z/rz chain)
                    psc = psum_ctx.tile([P, NCH], f32, tag="ctx")
                    for c in range(NCH):
                        for t in range(NT):
                            nc.tensor.matmul(
                                psc[:, c : c + 1],
                                lhsT=ets[t][:, c * P : (c + 1) * P],
                                rhs=E[:, t : t + 1],
                                start=(t == 0),
                                stop=(t == NT - 1),
                            )
                    ctx_sb = small.tile([P, NCH], f32, tag="ctx_sb")
                    nc.scalar.copy(ctx_sb, psc)
                    for c in range(NCH):
                        nc.sync.dma_start(out=out[b, c], in_=ctx_sb[:, c : c + 1])
                    continue

                Pw = small.tile([P, NT], f32, tag="P")
                srow = small.tile([P, 1], f32, tag="srow")
                nc.scalar.activation(
                    out=Pw, in_=E, func=Act.Exp, accum_out=srow
                )

                # Z = sum_p srow[p]  (cross-partition sum via PE)
                psz = psum_misc.tile([1, 1], f32, tag="z")
                nc.tensor.matmul(psz, lhsT=srow, rhs=ones_col, start=True, stop=True)
                z_sb = small.tile([1, 1], f32, tag="zsb")
                nc.scalar.copy(z_sb, psz)
                rz = small.tile([1, 1], f32, tag="rz")
                nc.vector.reciprocal(rz, z_sb)

                if STAGE == "exp":
                    psrz0 = psum_misc.tile([P, 1], f32, tag="rzb")
                    nc.tensor.matmul(
                        psrz0, lhsT=ones_row, rhs=rz, start=True, stop=True
                    )
                    rz_b0 = small.tile([P, 1], f32, tag="rzbs")
                    nc.scalar.copy(rz_b0, psrz0)
                    ctx_sb = small.tile([P, NCH], f32, tag="ctx_sb")
                    nc.scalar.activation(
                        out=ctx_sb, in_=Pw[:, :NCH], func=Act.Copy, scale=rz_b0
                    )
                    for c in range(NCH):
                        nc.sync.dma_start(out=out[b, c], in_=ctx_sb[:, c : c + 1])
                    continue

                if variant == "f32_stat":
                    # broadcast rz to all 128 partitions via K=1 matmul
                    psrz = psum_misc.tile([P, 1], f32, tag="rzb")
                    nc.tensor.matmul(
                        psrz, lhsT=ones_row, rhs=rz, start=True, stop=True
                    )
                    rz_b = small.tile([P, 1], f32, tag="rzbs")
                    nc.scalar.copy(rz_b, psrz)

                    # context[c*128+p] accumulated over 16 s-tiles; enc tile is
                    # the stationary operand so each matmul streams only N=1.
                    psc = psum_ctx.tile([P, NCH], f32, tag="ctx")
                    for c in range(NCH):
                        for t in range(NT):
                            nc.tensor.matmul(
                                psc[:, c : c + 1],
                                lhsT=ets[t][:, c * P : (c + 1) * P],
                                rhs=Pw[:, t : t + 1],
                                start=(t == 0),
                                stop=(t == NT - 1),
                            )
                    ctx_sb = small.tile([P, NCH], f32, tag="ctx_sb")
                    nc.scalar.activation(
                        out=ctx_sb, in_=psc, func=Act.Copy, scale=rz_b
                    )
                    for c in range(NCH):
                        nc.sync.dma_start(
                            out=out[b, c], in_=ctx_sb[:, c : c + 1]
                        )
                else:
                    # attn-stationary: N=512 moving operand, [1, 512] psum
                    # rows are contiguous. "f32r" bitcasts to the fast
                    # reduced-precision fp32 mode; "f32_mov" keeps exact fp32.
                    cast = (lambda ap: ap.bitcast(f32r)) if variant == "f32r" else (
                        lambda ap: ap
                    )
                    psc = psum_ctx.tile([1, H], f32, tag="ctx")
                    for half in range(2):
                        sl = slice(half * 512, (half + 1) * 512)
                        for t in range(NT):
                            nc.tensor.matmul(
                                psc[:, sl],
                                lhsT=cast(Pw[:, t : t + 1]),
                                rhs=cast(ets[t][:, sl]),
                                start=(t == 0),
                                stop=(t == NT - 1),
                            )
                    out_sb = small.tile([1, H], f32, tag="out_sb")
                    nc.scalar.activation(
                        out=out_sb, in_=psc, func=Act.Copy, scale=rz
                    )
                    nc.sync.dma_start(out=out[b : b + 1, :], in_=out_sb)

            if _rep is not None:
                _rep.__exit__(None, None, None)

    nc.compile()
    return nc


def _get_nc(variant):
    key = (variant, STAGE, DBG_BLOC, ENERGY, REPEAT, DMA_MODE, ENC_BUFS)
    if key not in _CACHE:
        _CACHE[key] = _build(variant)
    return _CACHE[key]


PROFILE = False
LAST_RESULTS = None


def kernel(hidden, encoder_outputs, W, b):
    global LAST_RESULTS
    from concourse import bass_utils

    variant = VARIANT
    nc = _get_nc(variant)

    enc = np.ascontiguousarray(np.asarray(encoder_outputs, dtype=np.float32))
    we = np.ascontiguousarray(
        np.asarray(W, dtype=np.float32)[H:, 0].reshape(1, H)
    )

    in_maps = [
        {"enc": enc[i * BLOC : (i + 1) * BLOC], "we": we} for i in range(NCORES)
    ]

    res = bass_utils.run_bass_kernel_spmd(
        nc,
        in_maps,
        core_ids=list(range(NCORES)),
        trace=PROFILE,
    )
    LAST_RESULTS = res

    outs = [res.results[i]["out"].reshape(BLOC, H) for i in range(NCORES)]
    return np.concatenate(outs, axis=0).astype(np.float32)
